# revision 11
# baseline (speedup 1.0000x reference)
"""Trainium2 Bass kernel for CircularUpsample2 (upfirdn2d up=2, circular pad).

out[b,c] = A @ x[b,c] @ B^T  per image, where A,B are (256,128) banded
circulant polyphase-upsample matrices built host-side from the 4x4 FIR
kernel (separable; the reference kernel is exactly rank-1 with bf16-exact
taps).

Device strategy (per core, pure data parallel over the 2048 b*c images):
  MM1: s = x^T A^T      (lhsT = x,  rhs = A^T)  -> PSUM (w, 2H)
  MM2: out_blk = s_blk^T B^T  for 2 row blocks  -> PSUM (rows, 2W)
No transposes needed anywhere. Fast path is memory-roofline oriented:
x is rounded to bf16 on host, matmuls run in bf16 (fp32 PSUM accumulate),
and the output is stored to HBM as bf16 and widened to fp32 on host,
halving both directions of HBM traffic (~1.5e-3 relative error, well
inside tolerance).
"""

import numpy as np
import ml_dtypes

import concourse.bass as bass
from concourse import bacc
import concourse.mybir as mybir
from concourse.tile import TileContext
from concourse.bass_utils import run_bass_kernel_spmd

BF16 = ml_dtypes.bfloat16
N_CORES = 8
H = W = 128
OH = OW = 256


# ---------------------------------------------------------------- host math
def _build_M(taps, n=H):
    """1-D polyphase factor (2n, n):
    out[2t]   = taps[2]*x[(t-2)%n] + taps[0]*x[(t-1)%n]
    out[2t+1] = taps[3]*x[(t-2)%n] + taps[1]*x[(t-1)%n]
    """
    M = np.zeros((2 * n, n), dtype=np.float32)
    t = np.arange(n)
    M[2 * t, (t - 2) % n] += taps[2]
    M[2 * t, (t - 1) % n] += taps[0]
    M[2 * t + 1, (t - 2) % n] += taps[3]
    M[2 * t + 1, (t - 1) % n] += taps[1]
    return M


def _factorize(k):
    """k (4,4) float32 -> list of (u, v) float32 with k = sum_r outer(u,v).

    Prefers an exact symmetric factorization for rank-1 PSD kernels so the
    taps stay exactly representable (the reference kernel's taps are
    0.25/0.75, exact in bf16).
    """
    k64 = k.astype(np.float64)
    U, S, Vt = np.linalg.svd(k64)
    rank = int(np.sum(S > 1e-7 * S[0]))
    if rank == 1:
        i = int(np.argmax(np.abs(np.diag(k64))))
        if k64[i, i] > 0:
            r = np.sqrt(k64[i, i])
            u = (k64[i, :] / r).astype(np.float32)
            if np.allclose(np.outer(u, u), k64, rtol=1e-6, atol=1e-9):
                return [(u, u.copy())]
        u = (U[:, 0] * S[0]).astype(np.float32)
        v = Vt[0, :].astype(np.float32)
        return [(u, v)]
    return [((U[:, r] * S[r]).astype(np.float32), Vt[r, :].astype(np.float32))
            for r in range(rank)]


# ---------------------------------------------------------------- bass build
def _build_nc_bf16(n_img, g_load=32, g_store=8):
    """bf16 fast path: A,B must be bf16-exact, rank 1.

    Input xb host-rounded to bf16 and transposed to (H, n_img, W) for
    contiguous loads; A row-permuted to [even; odd] so each partition's
    store chunk is the contiguous row pair (2p, 2p+1). Output stays bf16
    in HBM (host widens to fp32), halving store traffic. Two images per
    PSUM tile; the PE stream is software-pipelined one pair ahead (MM1 of
    pair i+1 is emitted before MM2 of pair i) so MM2 never stalls on the
    PSUM->SBUF cast copy. Copies split across engines: s-cast on Act,
    out-cast on DVE.
    """
    assert n_img % g_load == 0 and n_img % g_store == 0 and g_store % 2 == 0
    nc = bacc.Bacc("TRN2", target_bir_lowering=False)
    bf = mybir.dt.bfloat16
    f32 = mybir.dt.float32
    xb_d = nc.dram_tensor("xb", (H, n_img, W), bf, kind="ExternalInput")
    at_d = nc.dram_tensor("at", (H, OH), bf, kind="ExternalInput")
    bt_d = nc.dram_tensor("bt", (W, OW), bf, kind="ExternalInput")
    # partition-major output: out[p, img, (r j)] = image row 2p+r. Gives
    # 8KB-contiguous HBM runs per partition per store group (vs 1KB for
    # row-major), which is what gets the store stream to DMA line rate.
    # The host un-permutes with a single transpose+reshape.
    out_d = nc.dram_tensor("out", (128, n_img, 2 * OW), bf,
                           kind="ExternalOutput")

    with TileContext(nc) as tc:
        with tc.tile_pool(name="consts", bufs=1) as cpool, \
             tc.tile_pool(name="data", bufs=4) as pool, \
             tc.tile_pool(name="psum", bufs=1, space="PSUM") as ppool:
            at = cpool.tile([H, OH], bf)
            nc.scalar.dma_start(out=at, in_=at_d[:])
            bt = cpool.tile([W, OW], bf)
            nc.scalar.dma_start(out=bt, in_=bt_d[:])

            osb_tiles = {}

            sizes = [8, 8, 16] if n_img >= 64 else []
            rem = n_img - sum(sizes)
            sizes += [g_load] * (rem // g_load) + ([rem % g_load] if rem % g_load else [])
            load_groups = {}
            b0 = 0
            for sz in sizes:
                load_groups[b0] = sz
                b0 += sz

            def copy_eng(p0):
                # whole-pair engine assignment, alternating by pair parity:
                # exactly one PSUM-evict op per engine per pipeline slot,
                # minimal per-op fixed cost, deps naturally one slot apart.
                return nc.scalar if (p0 // 2) % 2 == 0 else nc.vector

            def evict(eng, dst, src):
                if eng is nc.scalar:
                    nc.scalar.copy(out=dst, in_=src)
                else:
                    nc.vector.tensor_copy(dst, src)

            def stage1(p0):
                """loads + MM1 + PSUM->SBUF cast for image pair (p0, p0+1)."""
                if p0 in load_groups:
                    g0, gsz = p0, load_groups[p0]
                    xg = pool.tile([128, gsz * W], bf, tag="xg", bufs=3,
                                   name=f"xg_{g0}")
                    # SWDGE (gpsimd) queue: keeps loads off the store ring
                    # (no FIFO coupling) and off the Act engine (busy with
                    # PSUM casts).
                    nc.gpsimd.dma_start(out=xg, in_=xb_d[:, g0:g0 + gsz, :])
                    stage1.xg, stage1.g0 = xg, g0
                xg, g0 = stage1.xg, stage1.g0
                s2_p = ppool.tile([128, 2 * OH], f32, tag="s2_p", bufs=2,
                                  name=f"s2_p_{p0}")
                for q in range(2):
                    gi = p0 + q - g0
                    nc.tensor.matmul(s2_p[:, q * OH:(q + 1) * OH],
                                     lhsT=xg[:, gi * W:(gi + 1) * W],
                                     rhs=at, start=True, stop=True)
                s_sb = pool.tile([128, 2 * OH], bf, tag="s_sb", bufs=3,
                                 name=f"s_sb_{p0}")
                evict(copy_eng(p0), s_sb, s2_p)
                return (s_sb,)

            def stage2(p0, s_sb):
                """MM2 + output cast copy + (on group tail) the store DMA."""
                s0 = (p0 // g_store) * g_store
                if s0 not in osb_tiles:
                    osb_tiles[s0] = pool.tile([128, g_store * 2 * OW], bf,
                                              tag="o_sb", bufs=3,
                                              name=f"o_sb_{s0}")
                o_sb = osb_tiles[s0]
                o2_p = ppool.tile([128, 4 * OW], f32, tag="o2_p", bufs=3,
                                  name=f"o2_p_{p0}")
                for q in range(2):
                    for r in range(2):
                        dst = o2_p[:, (q * 2 + r) * OW:(q * 2 + r + 1) * OW]
                        lo = q * OH + r * 128
                        nc.tensor.matmul(dst, lhsT=s_sb[:, lo:lo + 128],
                                         rhs=bt, start=True, stop=True)
                oq = (p0 - s0) * 2 * OW
                evict(copy_eng(p0), o_sb[:, oq:oq + 4 * OW], o2_p)
                last = s0 + g_store == n_img
                if last:
                    hg = g_store // 2
                    done = p0 + 2 - s0
                    for h0 in (0, hg):
                        if done == h0 + hg:
                            nc.sync.dma_start(
                                out=out_d[:, s0 + h0:s0 + h0 + hg, :],
                                in_=o_sb[:, h0 * 2 * OW:(h0 + hg) * 2 * OW])
                elif p0 + 2 == s0 + g_store:
                    nc.sync.dma_start(out=out_d[:, s0:s0 + g_store, :],
                                      in_=o_sb)
                    del osb_tiles[s0]

            prev = None
            for p0 in range(0, n_img, 2):
                cur = (p0, *stage1(p0))
                if prev is not None:
                    stage2(*prev)
                prev = cur
            stage2(*prev)
    nc.finalize()
    return nc


def _build_nc_fp32(n_img, n_terms, g_load=8, g_store=2):
    """general fp32 path, rank n_terms."""
    nc = bacc.Bacc("TRN2", target_bir_lowering=False)
    f32 = mybir.dt.float32
    R = n_terms
    x_d = nc.dram_tensor("x", (n_img, H, W), f32, kind="ExternalInput")
    at_d = nc.dram_tensor("at", (R, H, OH), f32, kind="ExternalInput")
    bt_d = nc.dram_tensor("bt", (R, W, OW), f32, kind="ExternalInput")
    out_d = nc.dram_tensor("out", (n_img, OH, OW), f32, kind="ExternalOutput")

    with TileContext(nc) as tc:
        with tc.tile_pool(name="consts", bufs=1) as cpool, \
             tc.tile_pool(name="data", bufs=4) as pool, \
             tc.tile_pool(name="psum", bufs=4, space="PSUM") as ppool:
            at = cpool.tile([H, R * OH], f32)
            nc.sync.dma_start(out=at.rearrange("p (r i) -> p r i", r=R), in_=at_d.rearrange("r h i -> h r i"))
            bt = cpool.tile([W, R * OW], f32)
            nc.sync.dma_start(out=bt.rearrange("p (r j) -> p r j", r=R), in_=bt_d.rearrange("r w j -> w r j"))

            for s0 in range(0, n_img, g_store):
                o_sb = pool.tile([128, g_store * 2 * OW], f32, tag="o_sb")
                for img in range(s0, s0 + g_store):
                    gi = img % g_load
                    if gi == 0:
                        g0 = img
                        xg = pool.tile([128, g_load * W], f32, tag="xg")
                        nc.sync.dma_start(
                            out=xg.rearrange("p (g w) -> p g w", g=g_load),
                            in_=x_d[g0:g0 + g_load].rearrange("g h w -> h g w"))
                    x_t = xg[:, gi * W:(gi + 1) * W]

                    s_p = ppool.tile([128, R * OH], f32, tag="s_p")
                    for r in range(R):
                        nc.tensor.matmul(s_p[:, r * OH:(r + 1) * OH], lhsT=x_t,
                                         rhs=at[:, r * OH:(r + 1) * OH],
                                         start=True, stop=True)
                    s_sb = pool.tile([128, R * OH], f32, tag="s_sb")
                    nc.scalar.copy(out=s_sb, in_=s_p)

                    oq = (img - s0) * 2 * OW
                    for blk in range(2):
                        o_p = ppool.tile([128, OW], f32, tag="o_p")
                        for r in range(R):
                            nc.tensor.matmul(
                                o_p,
                                lhsT=s_sb[:, r * OH + blk * 128: r * OH + (blk + 1) * 128],
                                rhs=bt[:, r * OW:(r + 1) * OW],
                                start=(r == 0), stop=(r == R - 1))
                        dst = o_sb[:, oq + blk * OW: oq + (blk + 1) * OW]
                        if blk == 0:
                            nc.vector.tensor_copy(dst, o_p)
                        else:
                            nc.scalar.copy(out=dst, in_=o_p)
                nc.sync.dma_start(
                    out=out_d[s0:s0 + g_store].rearrange("g (b p) j -> p g b j", b=2),
                    in_=o_sb.rearrange("p (g b j) -> p g b j", g=g_store, b=2))
    nc.finalize()
    return nc


_NC_CACHE = {}


def _get_nc(key, builder):
    if key not in _NC_CACHE:
        _NC_CACHE[key] = builder()
    return _NC_CACHE[key]


# ---------------------------------------------------------------- entry
def _run(x, kern, trace=False, n_cores=N_CORES):
    xf = np.ascontiguousarray(np.asarray(x, dtype=np.float32))
    k = np.asarray(kern, dtype=np.float32)
    b, c, h, w = xf.shape
    assert (h, w) == (H, W), (h, w)
    n_tot = b * c
    assert n_tot % n_cores == 0
    n_per = n_tot // n_cores
    imgs = xf.reshape(n_tot, h, w)

    terms = _factorize(k)
    fast = None
    if len(terms) == 1:
        A = _build_M(terms[0][0])
        Bm = _build_M(terms[0][1])
        if (np.array_equal(A.astype(BF16).astype(np.float32), A)
                and np.array_equal(Bm.astype(BF16).astype(np.float32), Bm)):
            fast = (A.astype(BF16), Bm.astype(BF16))

    if fast is not None:
        Ab, Bb = fast
        nc = _get_nc(("bf16", n_per), lambda: _build_nc_bf16(n_per))
        xb = imgs.astype(BF16)
        # permute A rows to [even; odd] so MM2 block r produces rows 2p+r
        Ap = np.concatenate([Ab[0::2], Ab[1::2]], axis=0)
        at = np.ascontiguousarray(Ap.T)
        bt = np.ascontiguousarray(Bb.T)
        in_maps = [
            {"xb": np.ascontiguousarray(
                 xb[i * n_per:(i + 1) * n_per].transpose(1, 0, 2)),
             "at": at, "bt": bt}
            for i in range(n_cores)
        ]
    else:
        R = len(terms)
        nc = _get_nc(("fp32", n_per, R), lambda: _build_nc_fp32(n_per, R))
        at = np.ascontiguousarray(
            np.stack([_build_M(u).T for (u, v) in terms]))
        bt = np.ascontiguousarray(
            np.stack([_build_M(v).T for (u, v) in terms]))
        in_maps = [
            {"x": imgs[i * n_per:(i + 1) * n_per], "at": at, "bt": bt}
            for i in range(n_cores)
        ]

    res = run_bass_kernel_spmd(nc, in_maps, list(range(n_cores)), trace=trace)
    if fast is not None:
        # device layout (128, n_img, 2*OW): partition p holds rows 2p, 2p+1
        out = np.concatenate(
            [np.asarray(res.results[i]["out"], dtype=np.float32)
                .transpose(1, 0, 2).reshape(n_per, OH, OW)
             for i in range(n_cores)], axis=0)
    else:
        out = np.concatenate([np.asarray(res.results[i]["out"],
                                         dtype=np.float32)
                              for i in range(n_cores)], axis=0)
    return out.reshape(b, c, OH, OW), res


def kernel(x, kernel):
    out, _ = _run(x, kernel, trace=False)
    return out



# revision 13
# speedup vs baseline: 1.1605x; 1.1605x over previous
"""Trainium2 Bass kernel for CircularUpsample2 (upfirdn2d up=2, circular pad).

out[b,c] = A @ x[b,c] @ B^T  per image, where A,B are (256,128) banded
circulant polyphase-upsample matrices built host-side from the 4x4 FIR
kernel (separable; the reference kernel is exactly rank-1 with bf16-exact
taps).

Device strategy (per core, pure data parallel over the 2048 b*c images):
  MM1: s = x^T A^T      (lhsT = x,  rhs = A^T)  -> PSUM (w, 2H)
  MM2: out_blk = s_blk^T B^T  for 2 row blocks  -> PSUM (rows, 2W)
No transposes needed anywhere. Fast path is memory-roofline oriented:
x is rounded to bf16 on host, matmuls run in bf16 (fp32 PSUM accumulate),
and the output is stored to HBM as bf16 and widened to fp32 on host,
halving both directions of HBM traffic (~1.5e-3 relative error, well
inside tolerance).
"""

import numpy as np
import ml_dtypes

import concourse.bass as bass
from concourse import bacc
import concourse.mybir as mybir
from concourse.tile import TileContext
from concourse.bass_utils import run_bass_kernel_spmd

BF16 = ml_dtypes.bfloat16
N_CORES = 8
H = W = 128
OH = OW = 256


# ---------------------------------------------------------------- host math
def _build_M(taps, n=H):
    """1-D polyphase factor (2n, n):
    out[2t]   = taps[2]*x[(t-2)%n] + taps[0]*x[(t-1)%n]
    out[2t+1] = taps[3]*x[(t-2)%n] + taps[1]*x[(t-1)%n]
    """
    M = np.zeros((2 * n, n), dtype=np.float32)
    t = np.arange(n)
    M[2 * t, (t - 2) % n] += taps[2]
    M[2 * t, (t - 1) % n] += taps[0]
    M[2 * t + 1, (t - 2) % n] += taps[3]
    M[2 * t + 1, (t - 1) % n] += taps[1]
    return M


def _factorize(k):
    """k (4,4) float32 -> list of (u, v) float32 with k = sum_r outer(u,v).

    Prefers an exact symmetric factorization for rank-1 PSD kernels so the
    taps stay exactly representable (the reference kernel's taps are
    0.25/0.75, exact in bf16).
    """
    k64 = k.astype(np.float64)
    U, S, Vt = np.linalg.svd(k64)
    rank = int(np.sum(S > 1e-7 * S[0]))
    if rank == 1:
        i = int(np.argmax(np.abs(np.diag(k64))))
        if k64[i, i] > 0:
            r = np.sqrt(k64[i, i])
            u = (k64[i, :] / r).astype(np.float32)
            if np.allclose(np.outer(u, u), k64, rtol=1e-6, atol=1e-9):
                return [(u, u.copy())]
        u = (U[:, 0] * S[0]).astype(np.float32)
        v = Vt[0, :].astype(np.float32)
        return [(u, v)]
    return [((U[:, r] * S[r]).astype(np.float32), Vt[r, :].astype(np.float32))
            for r in range(rank)]


# ---------------------------------------------------------------- bass build
def _build_nc_bf16(n_img, g_load=32, g_store=8):
    """bf16 fast path: A,B must be bf16-exact, rank 1.

    Input xb host-rounded to bf16 and transposed to (H, n_img, W) for
    contiguous loads; A row-permuted to [even; odd] so each partition's
    store chunk is the contiguous row pair (2p, 2p+1). Output stays bf16
    in HBM (host widens to fp32), halving store traffic. Two images per
    PSUM tile; the PE stream is software-pipelined one pair ahead (MM1 of
    pair i+1 is emitted before MM2 of pair i) so MM2 never stalls on the
    PSUM->SBUF cast copy. Copies split across engines: s-cast on Act,
    out-cast on DVE.
    """
    assert n_img % g_load == 0 and n_img % g_store == 0 and g_store % 2 == 0
    nc = bacc.Bacc("TRN2", target_bir_lowering=False)
    bf = mybir.dt.bfloat16
    f32 = mybir.dt.float32
    xb_d = nc.dram_tensor("xb", (H, n_img, W), bf, kind="ExternalInput")
    at_d = nc.dram_tensor("at", (H, OH), bf, kind="ExternalInput")
    bt_d = nc.dram_tensor("bt", (W, OW), bf, kind="ExternalInput")
    # partition-major output: out[p, img, (r j)] = image row 2p+r. Gives
    # 8KB-contiguous HBM runs per partition per store group (vs 1KB for
    # row-major), which is what gets the store stream to DMA line rate.
    # The host un-permutes with a single transpose+reshape.
    out_d = nc.dram_tensor("out", (128, n_img, 2 * OW), bf,
                           kind="ExternalOutput")

    with TileContext(nc) as tc:
        with tc.tile_pool(name="consts", bufs=1) as cpool, \
             tc.tile_pool(name="data", bufs=4) as pool, \
             tc.tile_pool(name="psum", bufs=1, space="PSUM") as ppool:
            at = cpool.tile([H, OH], bf)
            nc.scalar.dma_start(out=at, in_=at_d[:])
            bt = cpool.tile([W, OW], bf)
            nc.scalar.dma_start(out=bt, in_=bt_d[:])

            osb_tiles = {}

            sizes = [8, 8, 16] if n_img >= 64 else []
            rem = n_img - sum(sizes)
            sizes += [g_load] * (rem // g_load) + ([rem % g_load] if rem % g_load else [])
            load_groups = {}
            b0 = 0
            for sz in sizes:
                load_groups[b0] = sz
                b0 += sz

            def copy_eng(p0):
                # whole-pair engine assignment, alternating by pair parity:
                # exactly one PSUM-evict op per engine per pipeline slot,
                # minimal per-op fixed cost, deps naturally one slot apart.
                return nc.scalar if (p0 // 2) % 2 == 0 else nc.vector

            def evict(eng, dst, src):
                if eng is nc.scalar:
                    nc.scalar.copy(out=dst, in_=src)
                else:
                    nc.vector.tensor_copy(dst, src)

            def stage1(p0):
                """loads + MM1 + PSUM->SBUF cast for image pair (p0, p0+1)."""
                if p0 in load_groups:
                    g0, gsz = p0, load_groups[p0]
                    xg = pool.tile([128, gsz * W], bf, tag="xg", bufs=3,
                                   name=f"xg_{g0}")
                    # SWDGE (gpsimd) queue: keeps loads off the store ring
                    # (no FIFO coupling) and off the Act engine (busy with
                    # PSUM casts).
                    nc.gpsimd.dma_start(out=xg, in_=xb_d[:, g0:g0 + gsz, :])
                    stage1.xg, stage1.g0 = xg, g0
                xg, g0 = stage1.xg, stage1.g0
                s2_p = ppool.tile([128, 2 * OH], f32, tag="s2_p", bufs=2,
                                  name=f"s2_p_{p0}")
                for q in range(2):
                    gi = p0 + q - g0
                    nc.tensor.matmul(s2_p[:, q * OH:(q + 1) * OH],
                                     lhsT=xg[:, gi * W:(gi + 1) * W],
                                     rhs=at, start=True, stop=True)
                s_sb = pool.tile([128, 2 * OH], bf, tag="s_sb", bufs=4,
                                 name=f"s_sb_{p0}")
                evict(copy_eng(p0), s_sb, s2_p)
                return (s_sb,)

            def stage2(p0, s_sb):
                """MM2 + output cast copy + (on group tail) the store DMA."""
                s0 = (p0 // g_store) * g_store
                if s0 not in osb_tiles:
                    osb_tiles[s0] = pool.tile([128, g_store * 2 * OW], bf,
                                              tag="o_sb", bufs=3,
                                              name=f"o_sb_{s0}")
                o_sb = osb_tiles[s0]
                o2_p = ppool.tile([128, 4 * OW], f32, tag="o2_p", bufs=3,
                                  name=f"o2_p_{p0}")
                for q in range(2):
                    for r in range(2):
                        dst = o2_p[:, (q * 2 + r) * OW:(q * 2 + r + 1) * OW]
                        lo = q * OH + r * 128
                        nc.tensor.matmul(dst, lhsT=s_sb[:, lo:lo + 128],
                                         rhs=bt, start=True, stop=True)
                oq = (p0 - s0) * 2 * OW
                evict(copy_eng(p0), o_sb[:, oq:oq + 4 * OW], o2_p)
                last = s0 + g_store == n_img
                if last:
                    hg = g_store // 2
                    done = p0 + 2 - s0
                    for h0 in (0, hg):
                        if done == h0 + hg:
                            nc.sync.dma_start(
                                out=out_d[:, s0 + h0:s0 + h0 + hg, :],
                                in_=o_sb[:, h0 * 2 * OW:(h0 + hg) * 2 * OW])
                elif p0 + 2 == s0 + g_store:
                    nc.sync.dma_start(out=out_d[:, s0:s0 + g_store, :],
                                      in_=o_sb)
                    del osb_tiles[s0]

            # distance-2 software pipeline: MM2/evict of pair k are emitted
            # two iterations after its MM1/s-copy, so the ~650ns
            # s-copy -> MM2 -> out-copy latency chain is off every engine's
            # FIFO head by the time those ops are reached.
            from collections import deque
            pending = deque()
            for p0 in range(0, n_img, 2):
                pending.append((p0, *stage1(p0)))
                if len(pending) > 2:
                    stage2(*pending.popleft())
            while pending:
                stage2(*pending.popleft())
    nc.finalize()
    return nc


def _build_nc_fp32(n_img, n_terms, g_load=8, g_store=2):
    """general fp32 path, rank n_terms."""
    nc = bacc.Bacc("TRN2", target_bir_lowering=False)
    f32 = mybir.dt.float32
    R = n_terms
    x_d = nc.dram_tensor("x", (n_img, H, W), f32, kind="ExternalInput")
    at_d = nc.dram_tensor("at", (R, H, OH), f32, kind="ExternalInput")
    bt_d = nc.dram_tensor("bt", (R, W, OW), f32, kind="ExternalInput")
    out_d = nc.dram_tensor("out", (n_img, OH, OW), f32, kind="ExternalOutput")

    with TileContext(nc) as tc:
        with tc.tile_pool(name="consts", bufs=1) as cpool, \
             tc.tile_pool(name="data", bufs=4) as pool, \
             tc.tile_pool(name="psum", bufs=4, space="PSUM") as ppool:
            at = cpool.tile([H, R * OH], f32)
            nc.sync.dma_start(out=at.rearrange("p (r i) -> p r i", r=R), in_=at_d.rearrange("r h i -> h r i"))
            bt = cpool.tile([W, R * OW], f32)
            nc.sync.dma_start(out=bt.rearrange("p (r j) -> p r j", r=R), in_=bt_d.rearrange("r w j -> w r j"))

            for s0 in range(0, n_img, g_store):
                o_sb = pool.tile([128, g_store * 2 * OW], f32, tag="o_sb")
                for img in range(s0, s0 + g_store):
                    gi = img % g_load
                    if gi == 0:
                        g0 = img
                        xg = pool.tile([128, g_load * W], f32, tag="xg")
                        nc.sync.dma_start(
                            out=xg.rearrange("p (g w) -> p g w", g=g_load),
                            in_=x_d[g0:g0 + g_load].rearrange("g h w -> h g w"))
                    x_t = xg[:, gi * W:(gi + 1) * W]

                    s_p = ppool.tile([128, R * OH], f32, tag="s_p")
                    for r in range(R):
                        nc.tensor.matmul(s_p[:, r * OH:(r + 1) * OH], lhsT=x_t,
                                         rhs=at[:, r * OH:(r + 1) * OH],
                                         start=True, stop=True)
                    s_sb = pool.tile([128, R * OH], f32, tag="s_sb")
                    nc.scalar.copy(out=s_sb, in_=s_p)

                    oq = (img - s0) * 2 * OW
                    for blk in range(2):
                        o_p = ppool.tile([128, OW], f32, tag="o_p")
                        for r in range(R):
                            nc.tensor.matmul(
                                o_p,
                                lhsT=s_sb[:, r * OH + blk * 128: r * OH + (blk + 1) * 128],
                                rhs=bt[:, r * OW:(r + 1) * OW],
                                start=(r == 0), stop=(r == R - 1))
                        dst = o_sb[:, oq + blk * OW: oq + (blk + 1) * OW]
                        if blk == 0:
                            nc.vector.tensor_copy(dst, o_p)
                        else:
                            nc.scalar.copy(out=dst, in_=o_p)
                nc.sync.dma_start(
                    out=out_d[s0:s0 + g_store].rearrange("g (b p) j -> p g b j", b=2),
                    in_=o_sb.rearrange("p (g b j) -> p g b j", g=g_store, b=2))
    nc.finalize()
    return nc


_NC_CACHE = {}


def _get_nc(key, builder):
    if key not in _NC_CACHE:
        _NC_CACHE[key] = builder()
    return _NC_CACHE[key]


# ---------------------------------------------------------------- entry
def _run(x, kern, trace=False, n_cores=N_CORES):
    xf = np.ascontiguousarray(np.asarray(x, dtype=np.float32))
    k = np.asarray(kern, dtype=np.float32)
    b, c, h, w = xf.shape
    assert (h, w) == (H, W), (h, w)
    n_tot = b * c
    assert n_tot % n_cores == 0
    n_per = n_tot // n_cores
    imgs = xf.reshape(n_tot, h, w)

    terms = _factorize(k)
    fast = None
    if len(terms) == 1:
        A = _build_M(terms[0][0])
        Bm = _build_M(terms[0][1])
        if (np.array_equal(A.astype(BF16).astype(np.float32), A)
                and np.array_equal(Bm.astype(BF16).astype(np.float32), Bm)):
            fast = (A.astype(BF16), Bm.astype(BF16))

    if fast is not None:
        Ab, Bb = fast
        nc = _get_nc(("bf16", n_per), lambda: _build_nc_bf16(n_per))
        xb = imgs.astype(BF16)
        # permute A rows to [even; odd] so MM2 block r produces rows 2p+r
        Ap = np.concatenate([Ab[0::2], Ab[1::2]], axis=0)
        at = np.ascontiguousarray(Ap.T)
        bt = np.ascontiguousarray(Bb.T)
        in_maps = [
            {"xb": np.ascontiguousarray(
                 xb[i * n_per:(i + 1) * n_per].transpose(1, 0, 2)),
             "at": at, "bt": bt}
            for i in range(n_cores)
        ]
    else:
        R = len(terms)
        nc = _get_nc(("fp32", n_per, R), lambda: _build_nc_fp32(n_per, R))
        at = np.ascontiguousarray(
            np.stack([_build_M(u).T for (u, v) in terms]))
        bt = np.ascontiguousarray(
            np.stack([_build_M(v).T for (u, v) in terms]))
        in_maps = [
            {"x": imgs[i * n_per:(i + 1) * n_per], "at": at, "bt": bt}
            for i in range(n_cores)
        ]

    res = run_bass_kernel_spmd(nc, in_maps, list(range(n_cores)), trace=trace)
    if fast is not None:
        # device layout (128, n_img, 2*OW): partition p holds rows 2p, 2p+1
        out = np.concatenate(
            [np.asarray(res.results[i]["out"], dtype=np.float32)
                .transpose(1, 0, 2).reshape(n_per, OH, OW)
             for i in range(n_cores)], axis=0)
    else:
        out = np.concatenate([np.asarray(res.results[i]["out"],
                                         dtype=np.float32)
                              for i in range(n_cores)], axis=0)
    return out.reshape(b, c, OH, OW), res


def kernel(x, kernel):
    out, _ = _run(x, kernel, trace=False)
    return out



# revision 16
# speedup vs baseline: 1.2408x; 1.0692x over previous
"""Trainium2 Bass kernel for CircularUpsample2 (upfirdn2d up=2, circular pad).

out[b,c] = A @ x[b,c] @ B^T  per image, where A,B are (256,128) banded
circulant polyphase-upsample matrices built host-side from the 4x4 FIR
kernel (separable; the reference kernel is exactly rank-1 with bf16-exact
taps).

Device strategy (per core, pure data parallel over the 2048 b*c images):
  MM1: s = x^T A^T      (lhsT = x,  rhs = A^T)  -> PSUM (w, 2H)
  MM2: out_blk = s_blk^T B^T  for 2 row blocks  -> PSUM (rows, 2W)
No transposes needed anywhere. Fast path is memory-roofline oriented:
x is rounded to bf16 on host, matmuls run in bf16 (fp32 PSUM accumulate),
and the output is stored to HBM as bf16 and widened to fp32 on host,
halving both directions of HBM traffic (~1.5e-3 relative error, well
inside tolerance).
"""

import numpy as np
import ml_dtypes

import concourse.bass as bass
from concourse import bacc
import concourse.mybir as mybir
from concourse.tile import TileContext
from concourse.bass_utils import run_bass_kernel_spmd

BF16 = ml_dtypes.bfloat16
N_CORES = 8
H = W = 128
OH = OW = 256


# ---------------------------------------------------------------- host math
def _build_M(taps, n=H):
    """1-D polyphase factor (2n, n):
    out[2t]   = taps[2]*x[(t-2)%n] + taps[0]*x[(t-1)%n]
    out[2t+1] = taps[3]*x[(t-2)%n] + taps[1]*x[(t-1)%n]
    """
    M = np.zeros((2 * n, n), dtype=np.float32)
    t = np.arange(n)
    M[2 * t, (t - 2) % n] += taps[2]
    M[2 * t, (t - 1) % n] += taps[0]
    M[2 * t + 1, (t - 2) % n] += taps[3]
    M[2 * t + 1, (t - 1) % n] += taps[1]
    return M


def _factorize(k):
    """k (4,4) float32 -> list of (u, v) float32 with k = sum_r outer(u,v).

    Prefers an exact symmetric factorization for rank-1 PSD kernels so the
    taps stay exactly representable (the reference kernel's taps are
    0.25/0.75, exact in bf16).
    """
    k64 = k.astype(np.float64)
    U, S, Vt = np.linalg.svd(k64)
    rank = int(np.sum(S > 1e-7 * S[0]))
    if rank == 1:
        i = int(np.argmax(np.abs(np.diag(k64))))
        if k64[i, i] > 0:
            r = np.sqrt(k64[i, i])
            u = (k64[i, :] / r).astype(np.float32)
            if np.allclose(np.outer(u, u), k64, rtol=1e-6, atol=1e-9):
                return [(u, u.copy())]
        u = (U[:, 0] * S[0]).astype(np.float32)
        v = Vt[0, :].astype(np.float32)
        return [(u, v)]
    return [((U[:, r] * S[r]).astype(np.float32), Vt[r, :].astype(np.float32))
            for r in range(rank)]


# ---------------------------------------------------------------- bass build
def _build_nc_bf16(n_img, g_load=32, g_store=8):
    """bf16 fast path: A,B must be bf16-exact, rank 1.

    Input xb host-rounded to bf16 and transposed to (H, n_img, W) for
    contiguous loads; A row-permuted to [even; odd] so each partition's
    store chunk is the contiguous row pair (2p, 2p+1). Output stays bf16
    in HBM (host widens to fp32), halving store traffic. Two images per
    PSUM tile; the PE stream is software-pipelined one pair ahead (MM1 of
    pair i+1 is emitted before MM2 of pair i) so MM2 never stalls on the
    PSUM->SBUF cast copy. Copies split across engines: s-cast on Act,
    out-cast on DVE.
    """
    assert n_img % g_load == 0 and n_img % g_store == 0 and g_store % 2 == 0
    nc = bacc.Bacc("TRN2", target_bir_lowering=False)
    bf = mybir.dt.bfloat16
    f32 = mybir.dt.float32
    xb_d = nc.dram_tensor("xb", (H, n_img, W), bf, kind="ExternalInput")
    at_d = nc.dram_tensor("at", (H, OH), bf, kind="ExternalInput")
    bt_d = nc.dram_tensor("bt", (W, OW), bf, kind="ExternalInput")
    # partition-major output: out[p, img, (r j)] = image row 2p+r. Gives
    # 8KB-contiguous HBM runs per partition per store group (vs 1KB for
    # row-major), which is what gets the store stream to DMA line rate.
    # The host un-permutes with a single transpose+reshape.
    out_d = nc.dram_tensor("out", (128, n_img, 2 * OW), bf,
                           kind="ExternalOutput")

    with TileContext(nc) as tc:
        with tc.tile_pool(name="consts", bufs=1) as cpool, \
             tc.tile_pool(name="data", bufs=4) as pool, \
             tc.tile_pool(name="psum", bufs=1, space="PSUM") as ppool:
            at = cpool.tile([H, OH], bf)
            nc.scalar.dma_start(out=at, in_=at_d[:])
            bt = cpool.tile([W, OW], bf)
            nc.scalar.dma_start(out=bt, in_=bt_d[:])

            osb_tiles = {}

            sizes = [8, 8, 16] if n_img >= 64 else []
            rem = n_img - sum(sizes)
            sizes += [g_load] * (rem // g_load) + ([rem % g_load] if rem % g_load else [])
            group_starts = []
            b0 = 0
            for sz in sizes:
                group_starts.append(b0)
                b0 += sz
            group_of = {}
            for gi, g0 in enumerate(group_starts):
                for p in range(g0, g0 + sizes[gi], 2):
                    group_of[p] = gi
            xg_tiles = {}

            def issue_load(gi):
                g0, gsz = group_starts[gi], sizes[gi]
                xg = pool.tile([128, gsz * W], bf, tag="xg", bufs=3,
                               name=f"xg_{g0}")
                # SWDGE (gpsimd) queue: keeps loads off the store ring (no
                # FIFO coupling) and off the Act engine (busy with PSUM
                # casts).
                nc.gpsimd.dma_start(out=xg, in_=xb_d[:, g0:g0 + gsz, :])
                xg_tiles[gi] = xg

            def copy_eng(p0):
                # whole-pair engine assignment, alternating by pair parity:
                # exactly one PSUM-evict op per engine per pipeline slot,
                # minimal per-op fixed cost, deps naturally one slot apart.
                return nc.scalar if (p0 // 2) % 2 == 0 else nc.vector

            def evict(eng, dst, src):
                if eng is nc.scalar:
                    nc.scalar.copy(out=dst, in_=src)
                else:
                    nc.vector.tensor_copy(dst, src)

            def stage1(p0):
                """loads + MM1 + PSUM->SBUF cast for image pair (p0, p0+1)."""
                gi = group_of[p0]
                if p0 == group_starts[gi]:
                    # prefetch one group ahead so the SWDGE issue + transfer
                    # is never on the MM1 critical path (a late load stalls
                    # the PE >3.4us and re-throttles the HAM clock gate)
                    if gi == 0:
                        issue_load(0)
                    if gi + 1 < len(group_starts):
                        issue_load(gi + 1)
                    if gi - 2 in xg_tiles:
                        del xg_tiles[gi - 2]
                xg, g0 = xg_tiles[gi], group_starts[gi]
                s2_p = ppool.tile([128, 2 * OH], f32, tag="s2_p", bufs=2,
                                  name=f"s2_p_{p0}")
                for q in range(2):
                    qi = p0 + q - g0
                    nc.tensor.matmul(s2_p[:, q * OH:(q + 1) * OH],
                                     lhsT=xg[:, qi * W:(qi + 1) * W],
                                     rhs=at, start=True, stop=True)
                s_sb = pool.tile([128, 2 * OH], bf, tag="s_sb", bufs=4,
                                 name=f"s_sb_{p0}")
                evict(copy_eng(p0), s_sb, s2_p)
                return (s_sb,)

            def stage2(p0, s_sb):
                """MM2 + output cast copy + (on group tail) the store DMA."""
                s0 = (p0 // g_store) * g_store
                if s0 not in osb_tiles:
                    osb_tiles[s0] = pool.tile([128, g_store * 2 * OW], bf,
                                              tag="o_sb", bufs=4,
                                              name=f"o_sb_{s0}")
                o_sb = osb_tiles[s0]
                o2_p = ppool.tile([128, 4 * OW], f32, tag="o2_p", bufs=3,
                                  name=f"o2_p_{p0}")
                for q in range(2):
                    for r in range(2):
                        dst = o2_p[:, (q * 2 + r) * OW:(q * 2 + r + 1) * OW]
                        lo = q * OH + r * 128
                        nc.tensor.matmul(dst, lhsT=s_sb[:, lo:lo + 128],
                                         rhs=bt, start=True, stop=True)
                oq = (p0 - s0) * 2 * OW
                evict(copy_eng(p0), o_sb[:, oq:oq + 4 * OW], o2_p)
                last = s0 + g_store == n_img
                if last:
                    hg = g_store // 2
                    done = p0 + 2 - s0
                    for h0 in (0, hg):
                        if done == h0 + hg:
                            nc.sync.dma_start(
                                out=out_d[:, s0 + h0:s0 + h0 + hg, :],
                                in_=o_sb[:, h0 * 2 * OW:(h0 + hg) * 2 * OW])
                elif p0 + 2 == s0 + g_store:
                    nc.sync.dma_start(out=out_d[:, s0:s0 + g_store, :],
                                      in_=o_sb)
                    del osb_tiles[s0]

            # distance-2 software pipeline: MM2/evict of pair k are emitted
            # two iterations after its MM1/s-copy, so the ~650ns
            # s-copy -> MM2 -> out-copy latency chain is off every engine's
            # FIFO head by the time those ops are reached.
            from collections import deque
            pending = deque()
            for p0 in range(0, n_img, 2):
                pending.append((p0, *stage1(p0)))
                if len(pending) > 2:
                    stage2(*pending.popleft())
            while pending:
                stage2(*pending.popleft())
    nc.finalize()
    return nc


def _build_nc_fp32(n_img, n_terms, g_load=8, g_store=2):
    """general fp32 path, rank n_terms."""
    nc = bacc.Bacc("TRN2", target_bir_lowering=False)
    f32 = mybir.dt.float32
    R = n_terms
    x_d = nc.dram_tensor("x", (n_img, H, W), f32, kind="ExternalInput")
    at_d = nc.dram_tensor("at", (R, H, OH), f32, kind="ExternalInput")
    bt_d = nc.dram_tensor("bt", (R, W, OW), f32, kind="ExternalInput")
    out_d = nc.dram_tensor("out", (n_img, OH, OW), f32, kind="ExternalOutput")

    with TileContext(nc) as tc:
        with tc.tile_pool(name="consts", bufs=1) as cpool, \
             tc.tile_pool(name="data", bufs=4) as pool, \
             tc.tile_pool(name="psum", bufs=4, space="PSUM") as ppool:
            at = cpool.tile([H, R * OH], f32)
            nc.sync.dma_start(out=at.rearrange("p (r i) -> p r i", r=R), in_=at_d.rearrange("r h i -> h r i"))
            bt = cpool.tile([W, R * OW], f32)
            nc.sync.dma_start(out=bt.rearrange("p (r j) -> p r j", r=R), in_=bt_d.rearrange("r w j -> w r j"))

            for s0 in range(0, n_img, g_store):
                o_sb = pool.tile([128, g_store * 2 * OW], f32, tag="o_sb")
                for img in range(s0, s0 + g_store):
                    gi = img % g_load
                    if gi == 0:
                        g0 = img
                        xg = pool.tile([128, g_load * W], f32, tag="xg")
                        nc.sync.dma_start(
                            out=xg.rearrange("p (g w) -> p g w", g=g_load),
                            in_=x_d[g0:g0 + g_load].rearrange("g h w -> h g w"))
                    x_t = xg[:, gi * W:(gi + 1) * W]

                    s_p = ppool.tile([128, R * OH], f32, tag="s_p")
                    for r in range(R):
                        nc.tensor.matmul(s_p[:, r * OH:(r + 1) * OH], lhsT=x_t,
                                         rhs=at[:, r * OH:(r + 1) * OH],
                                         start=True, stop=True)
                    s_sb = pool.tile([128, R * OH], f32, tag="s_sb")
                    nc.scalar.copy(out=s_sb, in_=s_p)

                    oq = (img - s0) * 2 * OW
                    for blk in range(2):
                        o_p = ppool.tile([128, OW], f32, tag="o_p")
                        for r in range(R):
                            nc.tensor.matmul(
                                o_p,
                                lhsT=s_sb[:, r * OH + blk * 128: r * OH + (blk + 1) * 128],
                                rhs=bt[:, r * OW:(r + 1) * OW],
                                start=(r == 0), stop=(r == R - 1))
                        dst = o_sb[:, oq + blk * OW: oq + (blk + 1) * OW]
                        if blk == 0:
                            nc.vector.tensor_copy(dst, o_p)
                        else:
                            nc.scalar.copy(out=dst, in_=o_p)
                nc.sync.dma_start(
                    out=out_d[s0:s0 + g_store].rearrange("g (b p) j -> p g b j", b=2),
                    in_=o_sb.rearrange("p (g b j) -> p g b j", g=g_store, b=2))
    nc.finalize()
    return nc


_NC_CACHE = {}


def _get_nc(key, builder):
    if key not in _NC_CACHE:
        _NC_CACHE[key] = builder()
    return _NC_CACHE[key]


# ---------------------------------------------------------------- entry
def _run(x, kern, trace=False, n_cores=N_CORES):
    xf = np.ascontiguousarray(np.asarray(x, dtype=np.float32))
    k = np.asarray(kern, dtype=np.float32)
    b, c, h, w = xf.shape
    assert (h, w) == (H, W), (h, w)
    n_tot = b * c
    assert n_tot % n_cores == 0
    n_per = n_tot // n_cores
    imgs = xf.reshape(n_tot, h, w)

    terms = _factorize(k)
    fast = None
    if len(terms) == 1:
        A = _build_M(terms[0][0])
        Bm = _build_M(terms[0][1])
        if (np.array_equal(A.astype(BF16).astype(np.float32), A)
                and np.array_equal(Bm.astype(BF16).astype(np.float32), Bm)):
            fast = (A.astype(BF16), Bm.astype(BF16))

    if fast is not None:
        Ab, Bb = fast
        nc = _get_nc(("bf16", n_per), lambda: _build_nc_bf16(n_per))
        xb = imgs.astype(BF16)
        # permute A rows to [even; odd] so MM2 block r produces rows 2p+r
        Ap = np.concatenate([Ab[0::2], Ab[1::2]], axis=0)
        at = np.ascontiguousarray(Ap.T)
        bt = np.ascontiguousarray(Bb.T)
        in_maps = [
            {"xb": np.ascontiguousarray(
                 xb[i * n_per:(i + 1) * n_per].transpose(1, 0, 2)),
             "at": at, "bt": bt}
            for i in range(n_cores)
        ]
    else:
        R = len(terms)
        nc = _get_nc(("fp32", n_per, R), lambda: _build_nc_fp32(n_per, R))
        at = np.ascontiguousarray(
            np.stack([_build_M(u).T for (u, v) in terms]))
        bt = np.ascontiguousarray(
            np.stack([_build_M(v).T for (u, v) in terms]))
        in_maps = [
            {"x": imgs[i * n_per:(i + 1) * n_per], "at": at, "bt": bt}
            for i in range(n_cores)
        ]

    res = run_bass_kernel_spmd(nc, in_maps, list(range(n_cores)), trace=trace)
    if fast is not None:
        # device layout (128, n_img, 2*OW): partition p holds rows 2p, 2p+1
        out = np.concatenate(
            [np.asarray(res.results[i]["out"], dtype=np.float32)
                .transpose(1, 0, 2).reshape(n_per, OH, OW)
             for i in range(n_cores)], axis=0)
    else:
        out = np.concatenate([np.asarray(res.results[i]["out"],
                                         dtype=np.float32)
                              for i in range(n_cores)], axis=0)
    return out.reshape(b, c, OH, OW), res


def kernel(x, kernel):
    out, _ = _run(x, kernel, trace=False)
    return out



# revision 21
# speedup vs baseline: 1.3007x; 1.0482x over previous
"""Trainium2 Bass kernel for CircularUpsample2 (upfirdn2d up=2, circular pad).

out[b,c] = A @ x[b,c] @ B^T  per image, where A,B are (256,128) banded
circulant polyphase-upsample matrices built host-side from the 4x4 FIR
kernel (separable; the reference kernel is exactly rank-1 with bf16-exact
taps).

Device strategy (per core, pure data parallel over the 2048 b*c images):
  MM1: s = x^T A^T      (lhsT = x,  rhs = A^T)  -> PSUM (w, 2H)
  MM2: out_blk = s_blk^T B^T  for 2 row blocks  -> PSUM (rows, 2W)
No transposes needed anywhere. Fast path is memory-roofline oriented:
x is rounded to bf16 on host, matmuls run in bf16 (fp32 PSUM accumulate),
and the output is stored to HBM as bf16 and widened to fp32 on host,
halving both directions of HBM traffic (~1.5e-3 relative error, well
inside tolerance).
"""

import numpy as np
import ml_dtypes

import concourse.bass as bass
from concourse import bacc
import concourse.mybir as mybir
from concourse.tile import TileContext
from concourse.bass_utils import run_bass_kernel_spmd

BF16 = ml_dtypes.bfloat16
N_CORES = 8
H = W = 128
OH = OW = 256


# ---------------------------------------------------------------- host math
def _build_M(taps, n=H):
    """1-D polyphase factor (2n, n):
    out[2t]   = taps[2]*x[(t-2)%n] + taps[0]*x[(t-1)%n]
    out[2t+1] = taps[3]*x[(t-2)%n] + taps[1]*x[(t-1)%n]
    """
    M = np.zeros((2 * n, n), dtype=np.float32)
    t = np.arange(n)
    M[2 * t, (t - 2) % n] += taps[2]
    M[2 * t, (t - 1) % n] += taps[0]
    M[2 * t + 1, (t - 2) % n] += taps[3]
    M[2 * t + 1, (t - 1) % n] += taps[1]
    return M


def _factorize(k):
    """k (4,4) float32 -> list of (u, v) float32 with k = sum_r outer(u,v).

    Prefers an exact symmetric factorization for rank-1 PSD kernels so the
    taps stay exactly representable (the reference kernel's taps are
    0.25/0.75, exact in bf16).
    """
    k64 = k.astype(np.float64)
    U, S, Vt = np.linalg.svd(k64)
    rank = int(np.sum(S > 1e-7 * S[0]))
    if rank == 1:
        i = int(np.argmax(np.abs(np.diag(k64))))
        if k64[i, i] > 0:
            r = np.sqrt(k64[i, i])
            u = (k64[i, :] / r).astype(np.float32)
            if np.allclose(np.outer(u, u), k64, rtol=1e-6, atol=1e-9):
                return [(u, u.copy())]
        u = (U[:, 0] * S[0]).astype(np.float32)
        v = Vt[0, :].astype(np.float32)
        return [(u, v)]
    return [((U[:, r] * S[r]).astype(np.float32), Vt[r, :].astype(np.float32))
            for r in range(rank)]


# ---------------------------------------------------------------- bass build
def _build_nc_bf16(n_img, g_load=16, g_store=8):
    """bf16 fast path: A,B must be bf16-exact, rank 1.

    Input xb host-rounded to bf16 and transposed to (H, n_img, W) for
    contiguous loads; A row-permuted to [even; odd] so each partition's
    store chunk is the contiguous row pair (2p, 2p+1). Output stays bf16
    in HBM (host widens to fp32), halving store traffic. Two images per
    PSUM tile; the PE stream is software-pipelined one pair ahead (MM1 of
    pair i+1 is emitted before MM2 of pair i) so MM2 never stalls on the
    PSUM->SBUF cast copy. Copies split across engines: s-cast on Act,
    out-cast on DVE.
    """
    assert n_img % g_load == 0 and n_img % g_store == 0 and g_store % 2 == 0
    nc = bacc.Bacc("TRN2", target_bir_lowering=False)
    bf = mybir.dt.bfloat16
    f32 = mybir.dt.float32
    xb_d = nc.dram_tensor("xb", (H, n_img, W), bf, kind="ExternalInput")
    at_d = nc.dram_tensor("at", (H, OH), bf, kind="ExternalInput")
    bt_d = nc.dram_tensor("bt", (W, OW), bf, kind="ExternalInput")
    # partition-major output: out[p, img, (r j)] = image row 2p+r. Gives
    # 8KB-contiguous HBM runs per partition per store group (vs 1KB for
    # row-major), which is what gets the store stream to DMA line rate.
    # The host un-permutes with a single transpose+reshape.
    out_d = nc.dram_tensor("out", (128, n_img, 2 * OW), bf,
                           kind="ExternalOutput")

    with TileContext(nc) as tc:
        with tc.tile_pool(name="consts", bufs=1) as cpool, \
             tc.tile_pool(name="data", bufs=4) as pool, \
             tc.tile_pool(name="psum", bufs=1, space="PSUM") as ppool:
            at = cpool.tile([H, OH], bf)
            nc.scalar.dma_start(out=at, in_=at_d[:])
            bt = cpool.tile([W, OW], bf)
            nc.scalar.dma_start(out=bt, in_=bt_d[:])

            # HAM warmup: ~1.5us of dummy matmuls fired while the first
            # loads are still in flight, so the PE clock gate reaches 8/8
            # before (not 3.4us after) the real stream starts.
            warm = cpool.tile([128, 128], bf)
            nc.vector.memset(warm[:], 0.0)
            wp = ppool.tile([128, 2 * OH], f32, tag="s2_p", bufs=2,
                            name="warm")
            for _ in range(16):
                nc.tensor.matmul(wp[:, 0:128], lhsT=warm, rhs=warm,
                                 start=True, stop=True)

            osb_tiles = {}

            sizes = [8, 8] if n_img >= 64 else []
            rem = n_img - sum(sizes)
            sizes += [g_load] * (rem // g_load) + ([rem % g_load] if rem % g_load else [])
            group_starts = []
            b0 = 0
            for sz in sizes:
                group_starts.append(b0)
                b0 += sz
            group_of = {}
            for gi, g0 in enumerate(group_starts):
                for p in range(g0, g0 + sizes[gi], 2):
                    group_of[p] = gi
            xg_tiles = {}

            def issue_load(gi):
                g0, gsz = group_starts[gi], sizes[gi]
                xg = pool.tile([128, gsz * W], bf, tag="xg", bufs=3,
                               name=f"xg_{g0}")
                # First two groups ride the fast HWDGE sync ring (~0.6us
                # first byte) to shorten kernel startup; steady-state loads
                # use the SWDGE (gpsimd) queue, keeping them off the store
                # ring (no FIFO coupling) and off the busy Act engine.
                eng = nc.sync if gi < 2 else nc.gpsimd
                eng.dma_start(out=xg, in_=xb_d[:, g0:g0 + gsz, :])
                xg_tiles[gi] = xg

            def copy_eng(p0):
                # whole-pair engine assignment, alternating by pair parity:
                # exactly one PSUM-evict op per engine per pipeline slot,
                # minimal per-op fixed cost, deps naturally one slot apart.
                return nc.scalar if (p0 // 2) % 2 == 0 else nc.vector

            def evict(eng, dst, src):
                if eng is nc.scalar:
                    nc.scalar.copy(out=dst, in_=src)
                else:
                    nc.vector.tensor_copy(dst, src)

            def stage1(p0):
                """loads + MM1 + PSUM->SBUF cast for image pair (p0, p0+1)."""
                gi = group_of[p0]
                if p0 == group_starts[gi]:
                    # prefetch one group ahead so the SWDGE issue + transfer
                    # is never on the MM1 critical path (a late load stalls
                    # the PE >3.4us and re-throttles the HAM clock gate)
                    if gi == 0:
                        issue_load(0)
                    if gi + 1 < len(group_starts):
                        issue_load(gi + 1)
                    if gi - 2 in xg_tiles:
                        del xg_tiles[gi - 2]
                xg, g0 = xg_tiles[gi], group_starts[gi]
                s2_p = ppool.tile([128, 2 * OH], f32, tag="s2_p", bufs=2,
                                  name=f"s2_p_{p0}")
                for q in range(2):
                    qi = p0 + q - g0
                    nc.tensor.matmul(s2_p[:, q * OH:(q + 1) * OH],
                                     lhsT=xg[:, qi * W:(qi + 1) * W],
                                     rhs=at, start=True, stop=True)
                s_sb = pool.tile([128, 2 * OH], bf, tag="s_sb", bufs=4,
                                 name=f"s_sb_{p0}")
                evict(copy_eng(p0), s_sb, s2_p)
                return (s_sb,)

            def stage2(p0, s_sb):
                """MM2 + output cast copy + (on group tail) the store DMA."""
                s0 = (p0 // g_store) * g_store
                if s0 not in osb_tiles:
                    osb_tiles[s0] = pool.tile([128, g_store * 2 * OW], bf,
                                              tag="o_sb", bufs=4,
                                              name=f"o_sb_{s0}")
                o_sb = osb_tiles[s0]
                o2_p = ppool.tile([128, 4 * OW], f32, tag="o2_p", bufs=3,
                                  name=f"o2_p_{p0}")
                for q in range(2):
                    for r in range(2):
                        dst = o2_p[:, (q * 2 + r) * OW:(q * 2 + r + 1) * OW]
                        lo = q * OH + r * 128
                        nc.tensor.matmul(dst, lhsT=s_sb[:, lo:lo + 128],
                                         rhs=bt, start=True, stop=True)
                oq = (p0 - s0) * 2 * OW
                evict(copy_eng(p0), o_sb[:, oq:oq + 4 * OW], o2_p)
                last = s0 + g_store == n_img
                if last:
                    # drain the tail incrementally: store every pair as soon
                    # as its cast lands, so the final transfer is only 256KB
                    h0 = p0 - s0
                    nc.sync.dma_start(
                        out=out_d[:, s0 + h0:s0 + h0 + 2, :],
                        in_=o_sb[:, h0 * 2 * OW:(h0 + 2) * 2 * OW])
                elif p0 + 2 == s0 + g_store:
                    nc.sync.dma_start(out=out_d[:, s0:s0 + g_store, :],
                                      in_=o_sb)
                    del osb_tiles[s0]

            # distance-2 software pipeline: MM2/evict of pair k are emitted
            # two iterations after its MM1/s-copy, so the ~650ns
            # s-copy -> MM2 -> out-copy latency chain is off every engine's
            # FIFO head by the time those ops are reached.
            from collections import deque
            pending = deque()
            for p0 in range(0, n_img, 2):
                pending.append((p0, *stage1(p0)))
                if len(pending) > 2:
                    stage2(*pending.popleft())
            while pending:
                stage2(*pending.popleft())
    nc.finalize()
    return nc


def _build_nc_fp32(n_img, n_terms, g_load=8, g_store=2):
    """general fp32 path, rank n_terms."""
    nc = bacc.Bacc("TRN2", target_bir_lowering=False)
    f32 = mybir.dt.float32
    R = n_terms
    x_d = nc.dram_tensor("x", (n_img, H, W), f32, kind="ExternalInput")
    at_d = nc.dram_tensor("at", (R, H, OH), f32, kind="ExternalInput")
    bt_d = nc.dram_tensor("bt", (R, W, OW), f32, kind="ExternalInput")
    out_d = nc.dram_tensor("out", (n_img, OH, OW), f32, kind="ExternalOutput")

    with TileContext(nc) as tc:
        with tc.tile_pool(name="consts", bufs=1) as cpool, \
             tc.tile_pool(name="data", bufs=4) as pool, \
             tc.tile_pool(name="psum", bufs=4, space="PSUM") as ppool:
            at = cpool.tile([H, R * OH], f32)
            nc.sync.dma_start(out=at.rearrange("p (r i) -> p r i", r=R), in_=at_d.rearrange("r h i -> h r i"))
            bt = cpool.tile([W, R * OW], f32)
            nc.sync.dma_start(out=bt.rearrange("p (r j) -> p r j", r=R), in_=bt_d.rearrange("r w j -> w r j"))

            for s0 in range(0, n_img, g_store):
                o_sb = pool.tile([128, g_store * 2 * OW], f32, tag="o_sb")
                for img in range(s0, s0 + g_store):
                    gi = img % g_load
                    if gi == 0:
                        g0 = img
                        xg = pool.tile([128, g_load * W], f32, tag="xg")
                        nc.sync.dma_start(
                            out=xg.rearrange("p (g w) -> p g w", g=g_load),
                            in_=x_d[g0:g0 + g_load].rearrange("g h w -> h g w"))
                    x_t = xg[:, gi * W:(gi + 1) * W]

                    s_p = ppool.tile([128, R * OH], f32, tag="s_p")
                    for r in range(R):
                        nc.tensor.matmul(s_p[:, r * OH:(r + 1) * OH], lhsT=x_t,
                                         rhs=at[:, r * OH:(r + 1) * OH],
                                         start=True, stop=True)
                    s_sb = pool.tile([128, R * OH], f32, tag="s_sb")
                    nc.scalar.copy(out=s_sb, in_=s_p)

                    oq = (img - s0) * 2 * OW
                    for blk in range(2):
                        o_p = ppool.tile([128, OW], f32, tag="o_p")
                        for r in range(R):
                            nc.tensor.matmul(
                                o_p,
                                lhsT=s_sb[:, r * OH + blk * 128: r * OH + (blk + 1) * 128],
                                rhs=bt[:, r * OW:(r + 1) * OW],
                                start=(r == 0), stop=(r == R - 1))
                        dst = o_sb[:, oq + blk * OW: oq + (blk + 1) * OW]
                        if blk == 0:
                            nc.vector.tensor_copy(dst, o_p)
                        else:
                            nc.scalar.copy(out=dst, in_=o_p)
                nc.sync.dma_start(
                    out=out_d[s0:s0 + g_store].rearrange("g (b p) j -> p g b j", b=2),
                    in_=o_sb.rearrange("p (g b j) -> p g b j", g=g_store, b=2))
    nc.finalize()
    return nc


_NC_CACHE = {}


def _get_nc(key, builder):
    if key not in _NC_CACHE:
        _NC_CACHE[key] = builder()
    return _NC_CACHE[key]


# ---------------------------------------------------------------- entry
def _run(x, kern, trace=False, n_cores=N_CORES):
    xf = np.ascontiguousarray(np.asarray(x, dtype=np.float32))
    k = np.asarray(kern, dtype=np.float32)
    b, c, h, w = xf.shape
    assert (h, w) == (H, W), (h, w)
    n_tot = b * c
    assert n_tot % n_cores == 0
    n_per = n_tot // n_cores
    imgs = xf.reshape(n_tot, h, w)

    terms = _factorize(k)
    fast = None
    if len(terms) == 1:
        A = _build_M(terms[0][0])
        Bm = _build_M(terms[0][1])
        if (np.array_equal(A.astype(BF16).astype(np.float32), A)
                and np.array_equal(Bm.astype(BF16).astype(np.float32), Bm)):
            fast = (A.astype(BF16), Bm.astype(BF16))

    if fast is not None:
        Ab, Bb = fast
        nc = _get_nc(("bf16", n_per), lambda: _build_nc_bf16(n_per))
        xb = imgs.astype(BF16)
        # permute A rows to [even; odd] so MM2 block r produces rows 2p+r
        Ap = np.concatenate([Ab[0::2], Ab[1::2]], axis=0)
        at = np.ascontiguousarray(Ap.T)
        bt = np.ascontiguousarray(Bb.T)
        in_maps = [
            {"xb": np.ascontiguousarray(
                 xb[i * n_per:(i + 1) * n_per].transpose(1, 0, 2)),
             "at": at, "bt": bt}
            for i in range(n_cores)
        ]
    else:
        R = len(terms)
        nc = _get_nc(("fp32", n_per, R), lambda: _build_nc_fp32(n_per, R))
        at = np.ascontiguousarray(
            np.stack([_build_M(u).T for (u, v) in terms]))
        bt = np.ascontiguousarray(
            np.stack([_build_M(v).T for (u, v) in terms]))
        in_maps = [
            {"x": imgs[i * n_per:(i + 1) * n_per], "at": at, "bt": bt}
            for i in range(n_cores)
        ]

    res = run_bass_kernel_spmd(nc, in_maps, list(range(n_cores)), trace=trace)
    if fast is not None:
        # device layout (128, n_img, 2*OW): partition p holds rows 2p, 2p+1
        out = np.concatenate(
            [np.asarray(res.results[i]["out"], dtype=np.float32)
                .transpose(1, 0, 2).reshape(n_per, OH, OW)
             for i in range(n_cores)], axis=0)
    else:
        out = np.concatenate([np.asarray(res.results[i]["out"],
                                         dtype=np.float32)
                              for i in range(n_cores)], axis=0)
    return out.reshape(b, c, OH, OW), res


def kernel(x, kernel):
    out, _ = _run(x, kernel, trace=False)
    return out



# revision 22
# speedup vs baseline: 1.3307x; 1.0231x over previous
"""Trainium2 Bass kernel for CircularUpsample2 (upfirdn2d up=2, circular pad).

out[b,c] = A @ x[b,c] @ B^T  per image, where A,B are (256,128) banded
circulant polyphase-upsample matrices built host-side from the 4x4 FIR
kernel (separable; the reference kernel is exactly rank-1 with bf16-exact
taps).

Device strategy (per core, pure data parallel over the 2048 b*c images):
  MM1: s = x^T A^T      (lhsT = x,  rhs = A^T)  -> PSUM (w, 2H)
  MM2: out_blk = s_blk^T B^T  for 2 row blocks  -> PSUM (rows, 2W)
No transposes needed anywhere. Fast path is memory-roofline oriented:
x is rounded to bf16 on host, matmuls run in bf16 (fp32 PSUM accumulate),
and the output is stored to HBM as bf16 and widened to fp32 on host,
halving both directions of HBM traffic (~1.5e-3 relative error, well
inside tolerance).
"""

import numpy as np
import ml_dtypes

import concourse.bass as bass
from concourse import bacc
import concourse.mybir as mybir
from concourse.tile import TileContext
from concourse.bass_utils import run_bass_kernel_spmd

BF16 = ml_dtypes.bfloat16
N_CORES = 8
H = W = 128
OH = OW = 256


# ---------------------------------------------------------------- host math
def _build_M(taps, n=H):
    """1-D polyphase factor (2n, n):
    out[2t]   = taps[2]*x[(t-2)%n] + taps[0]*x[(t-1)%n]
    out[2t+1] = taps[3]*x[(t-2)%n] + taps[1]*x[(t-1)%n]
    """
    M = np.zeros((2 * n, n), dtype=np.float32)
    t = np.arange(n)
    M[2 * t, (t - 2) % n] += taps[2]
    M[2 * t, (t - 1) % n] += taps[0]
    M[2 * t + 1, (t - 2) % n] += taps[3]
    M[2 * t + 1, (t - 1) % n] += taps[1]
    return M


def _factorize(k):
    """k (4,4) float32 -> list of (u, v) float32 with k = sum_r outer(u,v).

    Prefers an exact symmetric factorization for rank-1 PSD kernels so the
    taps stay exactly representable (the reference kernel's taps are
    0.25/0.75, exact in bf16).
    """
    k64 = k.astype(np.float64)
    U, S, Vt = np.linalg.svd(k64)
    rank = int(np.sum(S > 1e-7 * S[0]))
    if rank == 1:
        i = int(np.argmax(np.abs(np.diag(k64))))
        if k64[i, i] > 0:
            r = np.sqrt(k64[i, i])
            u = (k64[i, :] / r).astype(np.float32)
            if np.allclose(np.outer(u, u), k64, rtol=1e-6, atol=1e-9):
                return [(u, u.copy())]
        u = (U[:, 0] * S[0]).astype(np.float32)
        v = Vt[0, :].astype(np.float32)
        return [(u, v)]
    return [((U[:, r] * S[r]).astype(np.float32), Vt[r, :].astype(np.float32))
            for r in range(rank)]


# ---------------------------------------------------------------- bass build
def _build_nc_bf16(n_img, g_load=16, g_store=8):
    """bf16 fast path: A,B must be bf16-exact, rank 1.

    Input xb host-rounded to bf16 and transposed to (H, n_img, W) for
    contiguous loads; A row-permuted to [even; odd] so each partition's
    store chunk is the contiguous row pair (2p, 2p+1). Output stays bf16
    in HBM (host widens to fp32), halving store traffic. Two images per
    PSUM tile; the PE stream is software-pipelined one pair ahead (MM1 of
    pair i+1 is emitted before MM2 of pair i) so MM2 never stalls on the
    PSUM->SBUF cast copy. Copies split across engines: s-cast on Act,
    out-cast on DVE.
    """
    assert n_img % g_load == 0 and n_img % g_store == 0 and g_store % 2 == 0
    nc = bacc.Bacc("TRN2", target_bir_lowering=False)
    bf = mybir.dt.bfloat16
    f32 = mybir.dt.float32
    xb_d = nc.dram_tensor("xb", (H, n_img, W), bf, kind="ExternalInput")
    at_d = nc.dram_tensor("at", (H, OH), bf, kind="ExternalInput")
    bt_d = nc.dram_tensor("bt", (W, OW), bf, kind="ExternalInput")
    # partition-major output: out[p, img, (r j)] = image row 2p+r. Gives
    # 8KB-contiguous HBM runs per partition per store group (vs 1KB for
    # row-major), which is what gets the store stream to DMA line rate.
    # The host un-permutes with a single transpose+reshape.
    out_d = nc.dram_tensor("out", (128, n_img, 2 * OW), bf,
                           kind="ExternalOutput")

    with TileContext(nc) as tc:
        with tc.tile_pool(name="consts", bufs=1) as cpool, \
             tc.tile_pool(name="data", bufs=4) as pool, \
             tc.tile_pool(name="psum", bufs=1, space="PSUM") as ppool:
            at = cpool.tile([H, OH], bf)
            nc.scalar.dma_start(out=at, in_=at_d[:])
            bt = cpool.tile([W, OW], bf)
            nc.scalar.dma_start(out=bt, in_=bt_d[:])

            # HAM warmup: ~1.5us of dummy matmuls fired while the first
            # loads are still in flight, so the PE clock gate reaches 8/8
            # before (not 3.4us after) the real stream starts.
            warm = cpool.tile([128, 128], bf)
            nc.vector.memset(warm[:], 0.0)
            wp = ppool.tile([128, 2 * OH], f32, tag="s2_p", bufs=2,
                            name="warm")
            for _ in range(16):
                nc.tensor.matmul(wp[:, 0:128], lhsT=warm, rhs=warm,
                                 start=True, stop=True)

            osb_tiles = {}

            sizes = [8, 8] if n_img >= 64 else []
            rem = n_img - sum(sizes)
            sizes += [g_load] * (rem // g_load) + ([rem % g_load] if rem % g_load else [])
            group_starts = []
            b0 = 0
            for sz in sizes:
                group_starts.append(b0)
                b0 += sz
            group_of = {}
            for gi, g0 in enumerate(group_starts):
                for p in range(g0, g0 + sizes[gi], 2):
                    group_of[p] = gi
            xg_tiles = {}

            def issue_load(gi):
                g0, gsz = group_starts[gi], sizes[gi]
                xg = pool.tile([128, gsz * W], bf, tag="xg", bufs=3,
                               name=f"xg_{g0}")
                # First two groups ride the fast HWDGE sync ring (~0.6us
                # first byte) to shorten kernel startup; steady-state loads
                # use the SWDGE (gpsimd) queue, keeping them off the store
                # ring (no FIFO coupling) and off the busy Act engine.
                eng = nc.sync if gi < 2 else nc.gpsimd
                eng.dma_start(out=xg, in_=xb_d[:, g0:g0 + gsz, :])
                xg_tiles[gi] = xg

            def copy_eng(p0):
                # whole-pair engine assignment, alternating by pair parity:
                # exactly one PSUM-evict op per engine per pipeline slot,
                # minimal per-op fixed cost, deps naturally one slot apart.
                return nc.scalar if (p0 // 2) % 2 == 0 else nc.vector

            def evict(eng, dst, src):
                if eng is nc.scalar:
                    nc.scalar.copy(out=dst, in_=src)
                else:
                    nc.vector.tensor_copy(dst, src)

            def stage1(p0):
                """loads + MM1 + PSUM->SBUF cast for image pair (p0, p0+1)."""
                gi = group_of[p0]
                if p0 == group_starts[gi]:
                    # prefetch one group ahead so the SWDGE issue + transfer
                    # is never on the MM1 critical path (a late load stalls
                    # the PE >3.4us and re-throttles the HAM clock gate)
                    if gi == 0:
                        issue_load(0)
                    if gi + 1 < len(group_starts):
                        issue_load(gi + 1)
                    if gi - 2 in xg_tiles:
                        del xg_tiles[gi - 2]
                xg, g0 = xg_tiles[gi], group_starts[gi]
                s2_p = ppool.tile([128, 2 * OH], f32, tag="s2_p", bufs=2,
                                  name=f"s2_p_{p0}")
                for q in range(2):
                    qi = p0 + q - g0
                    nc.tensor.matmul(s2_p[:, q * OH:(q + 1) * OH],
                                     lhsT=xg[:, qi * W:(qi + 1) * W],
                                     rhs=at, start=True, stop=True)
                s_sb = pool.tile([128, 2 * OH], bf, tag="s_sb", bufs=4,
                                 name=f"s_sb_{p0}")
                evict(copy_eng(p0), s_sb, s2_p)
                return (s_sb,)

            def stage2(p0, s_sb):
                """MM2 + output cast copy + (on group tail) the store DMA."""
                s0 = (p0 // g_store) * g_store
                if s0 not in osb_tiles:
                    osb_tiles[s0] = pool.tile([128, g_store * 2 * OW], bf,
                                              tag="o_sb", bufs=4,
                                              name=f"o_sb_{s0}")
                o_sb = osb_tiles[s0]
                o2_p = ppool.tile([128, 4 * OW], f32, tag="o2_p", bufs=3,
                                  name=f"o2_p_{p0}")
                for q in range(2):
                    for r in range(2):
                        dst = o2_p[:, (q * 2 + r) * OW:(q * 2 + r + 1) * OW]
                        lo = q * OH + r * 128
                        nc.tensor.matmul(dst, lhsT=s_sb[:, lo:lo + 128],
                                         rhs=bt, start=True, stop=True)
                oq = (p0 - s0) * 2 * OW
                evict(copy_eng(p0), o_sb[:, oq:oq + 4 * OW], o2_p)
                last = s0 + g_store == n_img
                if last:
                    # drain the tail incrementally: store every pair as soon
                    # as its cast lands, so the final transfer is only 256KB
                    h0 = p0 - s0
                    nc.sync.dma_start(
                        out=out_d[:, s0 + h0:s0 + h0 + 2, :],
                        in_=o_sb[:, h0 * 2 * OW:(h0 + 2) * 2 * OW])
                elif p0 + 2 == s0 + g_store:
                    nc.sync.dma_start(out=out_d[:, s0:s0 + g_store, :],
                                      in_=o_sb)
                    del osb_tiles[s0]

            # distance-2 software pipeline: MM2/evict of pair k are emitted
            # two iterations after its MM1/s-copy, so the ~650ns
            # s-copy -> MM2 -> out-copy latency chain is off every engine's
            # FIFO head by the time those ops are reached.
            from collections import deque
            pending = deque()
            for p0 in range(0, n_img, 2):
                pending.append((p0, *stage1(p0)))
                if len(pending) > 3:
                    stage2(*pending.popleft())
            while pending:
                stage2(*pending.popleft())
    nc.finalize()
    return nc


def _build_nc_fp32(n_img, n_terms, g_load=8, g_store=2):
    """general fp32 path, rank n_terms."""
    nc = bacc.Bacc("TRN2", target_bir_lowering=False)
    f32 = mybir.dt.float32
    R = n_terms
    x_d = nc.dram_tensor("x", (n_img, H, W), f32, kind="ExternalInput")
    at_d = nc.dram_tensor("at", (R, H, OH), f32, kind="ExternalInput")
    bt_d = nc.dram_tensor("bt", (R, W, OW), f32, kind="ExternalInput")
    out_d = nc.dram_tensor("out", (n_img, OH, OW), f32, kind="ExternalOutput")

    with TileContext(nc) as tc:
        with tc.tile_pool(name="consts", bufs=1) as cpool, \
             tc.tile_pool(name="data", bufs=4) as pool, \
             tc.tile_pool(name="psum", bufs=4, space="PSUM") as ppool:
            at = cpool.tile([H, R * OH], f32)
            nc.sync.dma_start(out=at.rearrange("p (r i) -> p r i", r=R), in_=at_d.rearrange("r h i -> h r i"))
            bt = cpool.tile([W, R * OW], f32)
            nc.sync.dma_start(out=bt.rearrange("p (r j) -> p r j", r=R), in_=bt_d.rearrange("r w j -> w r j"))

            for s0 in range(0, n_img, g_store):
                o_sb = pool.tile([128, g_store * 2 * OW], f32, tag="o_sb")
                for img in range(s0, s0 + g_store):
                    gi = img % g_load
                    if gi == 0:
                        g0 = img
                        xg = pool.tile([128, g_load * W], f32, tag="xg")
                        nc.sync.dma_start(
                            out=xg.rearrange("p (g w) -> p g w", g=g_load),
                            in_=x_d[g0:g0 + g_load].rearrange("g h w -> h g w"))
                    x_t = xg[:, gi * W:(gi + 1) * W]

                    s_p = ppool.tile([128, R * OH], f32, tag="s_p")
                    for r in range(R):
                        nc.tensor.matmul(s_p[:, r * OH:(r + 1) * OH], lhsT=x_t,
                                         rhs=at[:, r * OH:(r + 1) * OH],
                                         start=True, stop=True)
                    s_sb = pool.tile([128, R * OH], f32, tag="s_sb")
                    nc.scalar.copy(out=s_sb, in_=s_p)

                    oq = (img - s0) * 2 * OW
                    for blk in range(2):
                        o_p = ppool.tile([128, OW], f32, tag="o_p")
                        for r in range(R):
                            nc.tensor.matmul(
                                o_p,
                                lhsT=s_sb[:, r * OH + blk * 128: r * OH + (blk + 1) * 128],
                                rhs=bt[:, r * OW:(r + 1) * OW],
                                start=(r == 0), stop=(r == R - 1))
                        dst = o_sb[:, oq + blk * OW: oq + (blk + 1) * OW]
                        if blk == 0:
                            nc.vector.tensor_copy(dst, o_p)
                        else:
                            nc.scalar.copy(out=dst, in_=o_p)
                nc.sync.dma_start(
                    out=out_d[s0:s0 + g_store].rearrange("g (b p) j -> p g b j", b=2),
                    in_=o_sb.rearrange("p (g b j) -> p g b j", g=g_store, b=2))
    nc.finalize()
    return nc


_NC_CACHE = {}


def _get_nc(key, builder):
    if key not in _NC_CACHE:
        _NC_CACHE[key] = builder()
    return _NC_CACHE[key]


# ---------------------------------------------------------------- entry
def _run(x, kern, trace=False, n_cores=N_CORES):
    xf = np.ascontiguousarray(np.asarray(x, dtype=np.float32))
    k = np.asarray(kern, dtype=np.float32)
    b, c, h, w = xf.shape
    assert (h, w) == (H, W), (h, w)
    n_tot = b * c
    assert n_tot % n_cores == 0
    n_per = n_tot // n_cores
    imgs = xf.reshape(n_tot, h, w)

    terms = _factorize(k)
    fast = None
    if len(terms) == 1:
        A = _build_M(terms[0][0])
        Bm = _build_M(terms[0][1])
        if (np.array_equal(A.astype(BF16).astype(np.float32), A)
                and np.array_equal(Bm.astype(BF16).astype(np.float32), Bm)):
            fast = (A.astype(BF16), Bm.astype(BF16))

    if fast is not None:
        Ab, Bb = fast
        nc = _get_nc(("bf16", n_per), lambda: _build_nc_bf16(n_per))
        xb = imgs.astype(BF16)
        # permute A rows to [even; odd] so MM2 block r produces rows 2p+r
        Ap = np.concatenate([Ab[0::2], Ab[1::2]], axis=0)
        at = np.ascontiguousarray(Ap.T)
        bt = np.ascontiguousarray(Bb.T)
        in_maps = [
            {"xb": np.ascontiguousarray(
                 xb[i * n_per:(i + 1) * n_per].transpose(1, 0, 2)),
             "at": at, "bt": bt}
            for i in range(n_cores)
        ]
    else:
        R = len(terms)
        nc = _get_nc(("fp32", n_per, R), lambda: _build_nc_fp32(n_per, R))
        at = np.ascontiguousarray(
            np.stack([_build_M(u).T for (u, v) in terms]))
        bt = np.ascontiguousarray(
            np.stack([_build_M(v).T for (u, v) in terms]))
        in_maps = [
            {"x": imgs[i * n_per:(i + 1) * n_per], "at": at, "bt": bt}
            for i in range(n_cores)
        ]

    res = run_bass_kernel_spmd(nc, in_maps, list(range(n_cores)), trace=trace)
    if fast is not None:
        # device layout (128, n_img, 2*OW): partition p holds rows 2p, 2p+1
        out = np.concatenate(
            [np.asarray(res.results[i]["out"], dtype=np.float32)
                .transpose(1, 0, 2).reshape(n_per, OH, OW)
             for i in range(n_cores)], axis=0)
    else:
        out = np.concatenate([np.asarray(res.results[i]["out"],
                                         dtype=np.float32)
                              for i in range(n_cores)], axis=0)
    return out.reshape(b, c, OH, OW), res


def kernel(x, kernel):
    out, _ = _run(x, kernel, trace=False)
    return out

